# revision 10
# baseline (speedup 1.0000x reference)
"""BitLinear (ternary absmean-quantized linear) on 8 TRN2 NeuronCores.

Reference math (fp32):
    gamma = mean(|W|)
    Wq    = round(clip(W / (gamma + 1e-5), -1, 1))   # ternary {-1, 0, 1}
    out   = einsum('bsi,oi->bso', x, Wq)             # x @ Wq.T

Sharding: data-parallel over tokens. x [4,2048,4096] -> 8192 tokens, each
core owns 1024 of them and computes its full [1024, 4096] output slab with
no output collective. Every core needs the full quantized W; gamma (a global
scalar) is computed cooperatively: each core abs-sums 1/8 of W (512 of the
4096 output rows), a tiny [128,1] AllReduce combines the partials, and each
core then quantizes the full W on the fly while the TensorEngine consumes it.

Ternary quantization is exact in bf16, so the matmul runs in bf16
(x rounded to bf16, Wq in {-1,0,1} exactly) with fp32 PSUM accumulation.

Device kernel layout (per core):
    xT  [4096, 1024] bf16  - this core's x slab, transposed (K-major)
    WT  [4096, 4096] f32   - full W, transposed (in_features major), replicated
    Wg  [4096,  512] f32   - this core's gamma shard (= 512 columns of WT)
    out [1024, 4096] f32

Main loop: 8 N-chunks of 512 output features. Per chunk: stream 32 K-slabs
of WT, quantize each (|w| > t indicator on DVE, sign on ACT, product on DVE)
into a resident bf16 [128, 32, 512] chunk, then 8 m-tiles x 32 k-tiles of
128x128x512 bf16 matmuls accumulating in PSUM.
"""

import numpy as np
import ml_dtypes

NCORES = 8

# Full-problem dims (hardcoded per the harness contract).
B, S, D_IN, D_OUT = 4, 2048, 4096, 4096
M_TOTAL = B * S            # 8192 tokens
M_CORE = M_TOTAL // NCORES  # 1024 tokens per core

_COMPILED = None   # cached (nc, meta)
LAST_RESULTS = None  # BassKernelResults of the most recent run (for test.py)


FP8_PAIRS = 6  # KF: k-slab pairs (2*KF of 32 slabs) matmul'd in fp8 DoubleRow


def build_module(m_core=M_CORE, k=D_IN, n=D_OUT, ncores=NCORES, repeat=1,
                 use_collective=True, n_collectives=1,
                 quant_mode="full", w_dma_div=1, evict_engine="vector",
                 evict_direct=False, mt_div=1, fp8_pairs=FP8_PAIRS):
    """Build + compile the SPMD Bass module. Parametrized so a shrunken
    config can be validated in CoreSim. repeat>1 unrolls the whole kernel
    body multiple times inside one NEFF (for steady-state timing)."""
    import concourse.bass as bass  # noqa: F401
    import concourse.mybir as mybir
    import concourse.tile as tile
    from concourse import bacc
    from concourse import bass_isa

    f32 = mybir.dt.float32
    bf16 = mybir.dt.bfloat16
    f8 = mybir.dt.float8e4
    KT = k // 128            # k-tiles of 128
    MT = m_core // 128       # m-tiles of 128
    NCHUNK = 512             # output-feature chunk width
    NCHUNKS = n // NCHUNK
    NG = n // 8              # gamma shard width (columns of WT, 8-way shard)
    G_CHUNK = min(4, KT)     # k-tiles per gamma reduce chunk
    G_CHUNKS = KT // G_CHUNK
    N_ELEMS = float(k * n)
    KF = fp8_pairs           # k-slab pairs computed via fp8 DoubleRow
    KT8 = 2 * KF             # k-slabs in fp8 (k < KT8*128), rest bf16
    KTB = KT - KT8           # bf16 k-slabs

    nc = bacc.Bacc("TRN2", target_bir_lowering=False, debug=False,
                   num_devices=ncores)
    xT = None
    if KTB > 0:
        xT = nc.dram_tensor("xT", [KTB * 128, m_core], bf16,
                            kind="ExternalInput")
    xT8 = None
    if KF > 0:
        xT8 = nc.dram_tensor("xT8", [KT8 * 128, m_core], f8,
                             kind="ExternalInput")
    WT = nc.dram_tensor("WT", [k, n], f32, kind="ExternalInput")
    Wg = nc.dram_tensor("Wg", [k, NG], f32, kind="ExternalInput")
    out = nc.dram_tensor("out", [m_core, n], f32, kind="ExternalOutput")

    ts = bass.ts

    with tile.TileContext(nc) as tc:
        with (
            tc.tile_pool(name="xpool", bufs=1) as xpool,
            tc.tile_pool(name="gpool", bufs=2) as gpool,
            tc.tile_pool(name="wqpool", bufs=6) as wqpool,
            tc.tile_pool(name="wpool", bufs=16) as wpool,
            tc.tile_pool(name="spool", bufs=6) as spool,
            tc.tile_pool(name="opool", bufs=6) as opool,
            tc.tile_pool(name="small", bufs=2) as small,
            tc.tile_pool(name="pmain", bufs=8, space="PSUM") as pmain,
            tc.tile_pool(name="dram", bufs=2, space="DRAM") as dram,
        ):
          with tc.tile_pool(name="cpool", bufs=1) as cpool:
            bias_p = cpool.tile([128, 1], f32, name="bias_p")
            nc.gpsimd.memset(bias_p[:], 0.5e-5)
            bias_n = cpool.tile([128, 1], f32, name="bias_n")
            nc.gpsimd.memset(bias_n[:], -0.5e-5)
          for _rep in range(repeat):
            # ---- gamma: local abs-sum over this core's shard ----
            # Entirely on ACT + gpsimd (with its DMAs issued from the ACT
            # sequencer): these queues are idle during the main loop, so in
            # the repeat/steady-state case iteration i+1's whole gamma chain
            # (including the AllReduce) overlaps iteration i's matmuls
            # instead of queuing behind i's DVE/sync FIFOs.
            acc = small.tile([128, G_CHUNKS], f32)
            for j in range(G_CHUNKS):
                gsl = gpool.tile([128, G_CHUNK, NG], f32, tag="gsl")
                src = Wg[j * G_CHUNK * 128:(j + 1) * G_CHUNK * 128, :]
                # rep 0: sync queue -> gamma DMAs get strict head priority.
                # reps >0: ACT queue -> next iteration's gamma prefetch runs
                # under the current iteration's matmuls (sync FIFO is busy).
                geng = nc.sync if _rep == 0 else nc.scalar
                geng.dma_start(gsl[:], src.rearrange("(t p) c -> p t c", p=128))
                gscr = gpool.tile([128, G_CHUNK, NG], bf16, tag="gscr")
                nc.scalar.activation(
                    gscr[:], gsl[:], mybir.ActivationFunctionType.Abs,
                    accum_out=acc[:, j:j + 1])
            gpart = small.tile([128, 1], f32)
            gscr2 = small.tile([128, G_CHUNKS], bf16)
            nc.scalar.activation(
                gscr2[:], acc[:], mybir.ActivationFunctionType.Abs,
                accum_out=gpart[:])

            # ---- tiny AllReduce of per-partition partials ----
            gsum = small.tile([128, 1], f32)
            if ncores > 1 and use_collective:
                cin = dram.tile([128, 1], f32)
                nc.scalar.dma_start(cin[:], gpart[:])
                for ci in range(n_collectives):
                    cout = dram.tile([128, 1], f32, tag=f"cout{ci}",
                                     name=f"cout{ci}")
                    nc.gpsimd.collective_compute(
                        "AllReduce", mybir.AluOpType.add,
                        replica_groups=[list(range(ncores))],
                        ins=[cin[:].opt()], outs=[cout[:].opt()])
                    cin = cout
                nc.scalar.dma_start(gsum[:], cout[:])
            else:
                # timing/TimelineSim variant: no collective (gamma from the
                # local shard only -- numerically wrong, timing-equivalent)
                nc.scalar.copy(gsum[:], gpart[:])

            # sum across partitions, result broadcast to all partitions
            gtot = small.tile([128, 1], f32)
            nc.gpsimd.partition_all_reduce(
                gtot[:], gsum[:], channels=128, reduce_op=bass_isa.ReduceOp.add)

            # threshold t = 0.5 * (gamma + 1e-5)
            # Wq = (w > t) - (w < -t)  in {-1, 0, 1}
            tsb = small.tile([128, 1], f32)
            nc.scalar.activation(
                tsb[:], gtot[:], mybir.ActivationFunctionType.Identity,
                bias=bias_p[:], scale=0.5 / N_ELEMS)
            ntsb = small.tile([128, 1], f32)
            nc.scalar.activation(
                ntsb[:], gtot[:], mybir.ActivationFunctionType.Identity,
                bias=bias_n[:], scale=-0.5 / N_ELEMS)

            # ---- resident x: fp8 slabs [128, KT8, m] + bf16 [128, KTB, m] ----
            # Loaded lazily: slab kt's DMA is interleaved into chunk 0's
            # W stream (emitted just before W slab kt) so the first matmul
            # only waits for slab 0, not the whole 8.4 MB.
            xsb = xsb8 = xr = x8r = None
            if KTB > 0:
                xsb = xpool.tile([128, KTB, m_core], bf16)
                xr = xT[:, :].rearrange("(t p) m -> p t m", p=128)
            if KF > 0:
                xsb8 = xpool.tile([128, KT8, m_core], f8, tag="x8")
                x8r = xT8[:, :].rearrange("(t p) m -> p t m", p=128)

            # ---- main loop over output-feature chunks ----
            # kt-outer / mt-inner: each quantized W slab feeds the MT
            # parallel PSUM accumulation groups (one bank per m-tile)
            # immediately, so the PE ramps up right after the first slab is
            # quantized and each slab dies young (small wq pool).
            for c in range(NCHUNKS):
                ps = [pmain.tile([128, NCHUNK], f32, tag="ps", name=f"ps{mt}")
                      for mt in range(MT)]
                # -- fp8 DoubleRow part: k-slab pairs [2kp, 2kp+1] --
                for kp in range(KF):
                    if c == 0:
                        nc.sync.dma_start(xsb8[:, 2 * kp:2 * kp + 2, :],
                                          x8r[:, 2 * kp:2 * kp + 2, :])
                    wq8 = wqpool.tile([128, 2, NCHUNK], f8, tag="wq8")
                    for j in range(2):
                        kt = 2 * kp + j
                        wtmp = wpool.tile([128, NCHUNK], f32, tag="wtmp")
                        nc.sync.dma_start(
                            wtmp[:], WT[ts(kt, 128), ts(c, NCHUNK)])
                        neg = spool.tile([128, NCHUNK], bf16, tag="neg")
                        nc.vector.tensor_scalar(
                            neg[:], wtmp[:], ntsb[:], None,
                            mybir.AluOpType.is_lt)
                        nc.vector.scalar_tensor_tensor(
                            wq8[:, j, :], wtmp[:], tsb[:], neg[:],
                            mybir.AluOpType.is_gt, mybir.AluOpType.subtract)
                    for mt in range(MT // mt_div):
                        nc.tensor.matmul(
                            ps[mt][:], xsb8[:, 2 * kp:2 * kp + 2, ts(mt, 128)],
                            wq8[:], start=(kp == 0),
                            stop=(KTB == 0 and kp == KF - 1),
                            perf_mode=mybir.MatmulPerfMode.DoubleRow)
                # -- bf16 part: k-slabs KT8..KT-1 --
                wtmp = None
                for kt in range(KT8, KT):
                    kb = kt - KT8
                    if c == 0:
                        nc.sync.dma_start(xsb[:, kb, :], xr[:, kb, :])
                    if kb % w_dma_div == 0:
                        wtmp = wpool.tile([128, NCHUNK], f32, tag="wtmp")
                        nc.sync.dma_start(
                            wtmp[:], WT[ts(kt, 128), ts(c, NCHUNK)])
                    if quant_mode == "full":
                        neg = spool.tile([128, NCHUNK], bf16, tag="neg")
                        nc.vector.tensor_scalar(
                            neg[:], wtmp[:], ntsb[:], None,
                            mybir.AluOpType.is_lt)
                        wqt = wqpool.tile([128, NCHUNK], bf16, tag="wq")
                        nc.vector.scalar_tensor_tensor(
                            wqt[:], wtmp[:], tsb[:], neg[:],
                            mybir.AluOpType.is_gt, mybir.AluOpType.subtract)
                    elif quant_mode == "copy":
                        wqt = wqpool.tile([128, NCHUNK], bf16, tag="wq")
                        nc.vector.tensor_copy(wqt[:], wtmp[:])
                    elif quant_mode == "none":
                        wqt = wtmp
                    for mt in range(MT // mt_div):
                        nc.tensor.matmul(
                            ps[mt][:], xsb[:, kb, ts(mt, 128)], wqt[:],
                            start=(KF == 0 and kt == KT8),
                            stop=(kt == KT - 1))
                for mt in range(MT // mt_div):
                    if evict_direct:
                        nc.sync.dma_start(out[ts(mt, 128), ts(c, NCHUNK)],
                                          ps[mt][:])
                        continue
                    osb = opool.tile([128, NCHUNK], f32, tag="osb")
                    eng = nc.vector if evict_engine == "vector" else nc.scalar
                    if evict_engine == "vector":
                        eng.tensor_copy(osb[:], ps[mt][:])
                    else:
                        eng.copy(osb[:], ps[mt][:])
                    nc.sync.dma_start(out[ts(mt, 128), ts(c, NCHUNK)], osb[:])

    nc.compile()
    meta = dict(m_core=m_core, k=k, n=n, ncores=ncores, NG=NG)
    return nc, meta


def _get_compiled():
    global _COMPILED
    if _COMPILED is None:
        _COMPILED = build_module()
    return _COMPILED


def make_in_maps(x, W, m_core=M_CORE, ncores=NCORES, fp8_pairs=FP8_PAIRS):
    """Host-side shard prep. x [B,S,D_IN] f32, W [D_OUT,D_IN] f32.
    k < 256*fp8_pairs ships as fp8e4 (consumed by DoubleRow matmuls),
    the rest as bf16."""
    k = W.shape[1]
    n = W.shape[0]
    ng = n // ncores
    k8 = 256 * fp8_pairs
    x2 = np.asarray(x, dtype=np.float32).reshape(-1, k)
    xb = x2[:, k8:].astype(ml_dtypes.bfloat16)
    x8 = x2[:, :k8].astype(ml_dtypes.float8_e4m3)
    WT = np.ascontiguousarray(np.asarray(W, dtype=np.float32).T)  # [k, n]
    in_maps = []
    for c in range(ncores):
        sl = slice(c * m_core, (c + 1) * m_core)
        Wgc = np.ascontiguousarray(WT[:, c * ng:(c + 1) * ng])
        m = {"WT": WT, "Wg": Wgc}
        if k8 < k:
            m["xT"] = np.ascontiguousarray(xb[sl].T)
        if k8 > 0:
            m["xT8"] = np.ascontiguousarray(x8[sl].T)
        in_maps.append(m)
    return in_maps


def kernel(input, W):
    """Full inputs in, full output out. Shards internally across 8 cores."""
    global LAST_RESULTS
    from concourse import bass_utils

    nc, meta = _get_compiled()
    in_maps = make_in_maps(input, W)
    res = bass_utils.run_bass_kernel_spmd(
        nc, in_maps, core_ids=list(range(NCORES)))
    LAST_RESULTS = res
    out = np.concatenate([res.results[c]["out"] for c in range(NCORES)], axis=0)
    return out.reshape(B, S, D_OUT).astype(np.float32)



# revision 22
# speedup vs baseline: 140.8075x; 140.8075x over previous
"""BitLinear (ternary absmean-quantized linear) on 8 TRN2 NeuronCores.

Reference math (fp32):
    gamma = mean(|W|)
    Wq    = round(clip(W / (gamma + 1e-5), -1, 1))   # ternary {-1, 0, 1}
    out   = einsum('bsi,oi->bso', x, Wq)             # x @ Wq.T

Sharding: data-parallel over tokens. x [4,2048,4096] -> 8192 tokens, each
core owns 1024 of them and computes its full [1024, 4096] output slab with
no output collective. Every core needs the full quantized W; gamma (a global
scalar) is computed cooperatively: each core abs-sums 1/8 of W (512 of the
4096 output rows), a tiny [128,1] AllReduce combines the partials, and each
core then quantizes the full W on the fly while the TensorEngine consumes it.

Precision/speed split (the PE on these parts sustains ~2.0 GHz; bf16
roofline for the 34.4 GFLOP/core is ~528 us): ternary Wq is exact in both
bf16 and fp8e4, so the matmul runs as a hybrid --
  * k < 2048 (16 of 32 k-slabs): x in fp8e4 (e4m3), Wq in fp8e4, matmul'd
    as 8 DoubleRow pairs per chunk (256-deep contraction per instruction,
    ~2x bf16 throughput). e4m3 rounding of half of x costs 1.88e-2 rel
    error (gate: 2e-2); all other error sources are ~1.7e-3.
  * k >= 2048: x in bf16, Wq in bf16, standard 128x128x512 matmuls.
PSUM accumulates everything in fp32. PE time ~398 us/core + ~15 us stalls.

Device kernel layout (per core):
    xT  [2048, 1024] bf16  - bf16 half of this core's x slab, K-major
    xT8 [2048, 1024] f8e4  - fp8 half (k < 2048), K-major
    WT  [4096, 4096] f32   - full W, transposed (in_features major), replicated
    Wg  [4096,  512] f32   - this core's gamma shard (= 512 columns of WT)
    out [1024, 4096] f32

Main loop: 8 N-chunks of 512 output features. Per chunk: bf16 k-slabs
first (W slab DMA -> DVE quant [is_lt + is_gt/subtract] -> 8 m-tile
matmuls), then the fp8 DoubleRow pairs (W pair staged in one [128,2,512]
tile, one wide DVE quant pair, 8 DR matmuls). bf16-first ordering lets the
DVE quant stream build a lead before the DR burst consumes W 2x faster.
Evicted outputs DMA out one chunk late so their semaphore waits never
stall the next chunk's W stream on the sync queue.
"""

import numpy as np
import ml_dtypes

NCORES = 8

# Full-problem dims (hardcoded per the harness contract).
B, S, D_IN, D_OUT = 4, 2048, 4096, 4096
M_TOTAL = B * S            # 8192 tokens
M_CORE = M_TOTAL // NCORES  # 1024 tokens per core

_COMPILED = None   # cached (nc, meta)
LAST_RESULTS = None  # BassKernelResults of the most recent run (for test.py)


FP8_PAIRS = 8  # KF: k-slab pairs (2*KF of 32 slabs) matmul'd in fp8 DoubleRow


def build_module(m_core=M_CORE, k=D_IN, n=D_OUT, ncores=NCORES, repeat=1,
                 use_collective=True, n_collectives=1,
                 quant_mode="full", w_dma_div=1, evict_engine="vector",
                 mt_div=1, fp8_pairs=FP8_PAIRS):
    """Build + compile the SPMD Bass module. Parametrized so a shrunken
    config can be validated in CoreSim. repeat>1 unrolls the whole kernel
    body multiple times inside one NEFF (for steady-state timing)."""
    import concourse.bass as bass  # noqa: F401
    import concourse.mybir as mybir
    import concourse.tile as tile
    from concourse import bacc
    from concourse import bass_isa

    f32 = mybir.dt.float32
    bf16 = mybir.dt.bfloat16
    f8 = mybir.dt.float8e4
    KT = k // 128            # k-tiles of 128
    MT = m_core // 128       # m-tiles of 128
    NCHUNK = 512             # output-feature chunk width
    NCHUNKS = n // NCHUNK
    NG = n // 8              # gamma shard width (columns of WT, 8-way shard)
    G_CHUNK = min(4, KT)     # k-tiles per gamma reduce chunk
    G_CHUNKS = KT // G_CHUNK
    N_ELEMS = float(k * n)
    KF = fp8_pairs           # k-slab pairs computed via fp8 DoubleRow
    KT8 = 2 * KF             # k-slabs in fp8 (k < KT8*128), rest bf16
    KTB = KT - KT8           # bf16 k-slabs

    nc = bacc.Bacc("TRN2", target_bir_lowering=False, debug=False,
                   num_devices=ncores)
    xT = None
    if KTB > 0:
        xT = nc.dram_tensor("xT", [KTB * 128, m_core], bf16,
                            kind="ExternalInput")
    xT8 = None
    if KF > 0:
        xT8 = nc.dram_tensor("xT8", [KT8 * 128, m_core], f8,
                             kind="ExternalInput")
    WT = nc.dram_tensor("WT", [k, n], f32, kind="ExternalInput")
    Wg = nc.dram_tensor("Wg", [k, NG], f32, kind="ExternalInput")
    out = nc.dram_tensor("out", [m_core, n], f32, kind="ExternalOutput")

    ts = bass.ts

    with tile.TileContext(nc) as tc:
        with (
            tc.tile_pool(name="xpool", bufs=1) as xpool,
            tc.tile_pool(name="x8pool", bufs=2) as x8pool,
            tc.tile_pool(name="gpool", bufs=2) as gpool,
            tc.tile_pool(name="wqpool", bufs=6) as wqpool,
            tc.tile_pool(name="wq8pool", bufs=10) as wq8pool,
            tc.tile_pool(name="wpool", bufs=10) as wpool,
            tc.tile_pool(name="wppool", bufs=4) as wppool,
            tc.tile_pool(name="spool", bufs=6) as spool,
            tc.tile_pool(name="opool", bufs=14) as opool,
            tc.tile_pool(name="small", bufs=2) as small,
            tc.tile_pool(name="pmain", bufs=8, space="PSUM") as pmain,
            tc.tile_pool(name="dram", bufs=2, space="DRAM") as dram,
        ):
          with tc.tile_pool(name="cpool", bufs=1) as cpool:
            bias_p = cpool.tile([128, 1], f32, name="bias_p")
            nc.gpsimd.memset(bias_p[:], 0.5e-5)
            bias_n = cpool.tile([128, 1], f32, name="bias_n")
            nc.gpsimd.memset(bias_n[:], -0.5e-5)
          pending = []   # (osb, mt, c) evicted outputs awaiting DRAM DMA;
          # drained inside the *next* chunk's (or next repeat's) k-loop so
          # the out-DMA semaphore waits never stall the sync-queue W stream.
          for _rep in range(repeat):
            # ---- gamma: local abs-sum over this core's shard ----
            # Entirely on ACT + gpsimd (with its DMAs issued from the ACT
            # sequencer): these queues are idle during the main loop, so in
            # the repeat/steady-state case iteration i+1's whole gamma chain
            # (including the AllReduce) overlaps iteration i's matmuls
            # instead of queuing behind i's DVE/sync FIFOs.
            acc = small.tile([128, G_CHUNKS], f32)
            for j in range(G_CHUNKS):
                gsl = gpool.tile([128, G_CHUNK, NG], f32, tag="gsl")
                src = Wg[j * G_CHUNK * 128:(j + 1) * G_CHUNK * 128, :]
                # rep 0: sync queue -> gamma DMAs get strict head priority.
                # reps >0: ACT queue -> next iteration's gamma prefetch runs
                # under the current iteration's matmuls (sync FIFO is busy).
                geng = nc.sync if _rep == 0 else nc.scalar
                geng.dma_start(gsl[:], src.rearrange("(t p) c -> p t c", p=128))
                gscr = gpool.tile([128, G_CHUNK, NG], bf16, tag="gscr")
                nc.scalar.activation(
                    gscr[:], gsl[:], mybir.ActivationFunctionType.Abs,
                    accum_out=acc[:, j:j + 1])
            gpart = small.tile([128, 1], f32)
            gscr2 = small.tile([128, G_CHUNKS], bf16)
            nc.scalar.activation(
                gscr2[:], acc[:], mybir.ActivationFunctionType.Abs,
                accum_out=gpart[:])

            # ---- tiny AllReduce of per-partition partials ----
            gsum = small.tile([128, 1], f32)
            if ncores > 1 and use_collective:
                cin = dram.tile([128, 1], f32)
                nc.scalar.dma_start(cin[:], gpart[:])
                for ci in range(n_collectives):
                    cout = dram.tile([128, 1], f32, tag=f"cout{ci}",
                                     name=f"cout{ci}")
                    nc.gpsimd.collective_compute(
                        "AllReduce", mybir.AluOpType.add,
                        replica_groups=[list(range(ncores))],
                        ins=[cin[:].opt()], outs=[cout[:].opt()])
                    cin = cout
                nc.scalar.dma_start(gsum[:], cout[:])
            else:
                # timing/TimelineSim variant: no collective (gamma from the
                # local shard only -- numerically wrong, timing-equivalent)
                nc.scalar.copy(gsum[:], gpart[:])

            # sum across partitions, result broadcast to all partitions
            gtot = small.tile([128, 1], f32)
            nc.gpsimd.partition_all_reduce(
                gtot[:], gsum[:], channels=128, reduce_op=bass_isa.ReduceOp.add)

            # threshold t = 0.5 * (gamma + 1e-5)
            # Wq = (w > t) - (w < -t)  in {-1, 0, 1}
            tsb = small.tile([128, 1], f32)
            nc.scalar.activation(
                tsb[:], gtot[:], mybir.ActivationFunctionType.Identity,
                bias=bias_p[:], scale=0.5 / N_ELEMS)
            ntsb = small.tile([128, 1], f32)
            nc.scalar.activation(
                ntsb[:], gtot[:], mybir.ActivationFunctionType.Identity,
                bias=bias_n[:], scale=-0.5 / N_ELEMS)

            # ---- resident x: fp8 slabs [128, KT8, m] + bf16 [128, KTB, m] ----
            # Loaded lazily: slab kt's DMA is interleaved into chunk 0's
            # W stream (emitted just before W slab kt) so the first matmul
            # only waits for slab 0, not the whole 8.4 MB.
            xsb = xsb8 = xr = x8r = None
            if KTB > 0:
                xsb = xpool.tile([128, KTB, m_core], bf16)
                xr = xT[:, :].rearrange("(t p) m -> p t m", p=128)
            if KF > 0:
                xsb8 = x8pool.tile([128, KT8, m_core], f8, tag="x8")
                x8r = xT8[:, :].rearrange("(t p) m -> p t m", p=128)

            # ---- main loop over output-feature chunks ----
            # kt-outer / mt-inner: each quantized W slab feeds the MT
            # parallel PSUM accumulation groups (one bank per m-tile)
            # immediately, so the PE ramps up right after the first slab is
            # quantized and each slab dies young (small wq pool).
            for c in range(NCHUNKS):
                ps = [pmain.tile([128, NCHUNK], f32, tag="ps", name=f"ps{mt}")
                      for mt in range(MT)]
                # -- bf16 part first: k-slabs KT8..KT-1. The DVE quant
                # stream runs ~1.3us/slab faster than the PE consumes bf16
                # slabs, so by the time the DoubleRow burst starts (which
                # consumes W 2x faster than DVE quantizes), a lead of
                # quantized fp8 pairs is already banked in SBUF. --
                wtmp = None
                for kt in range(KT8, KT):
                    kb = kt - KT8
                    if c == 0:
                        nc.sync.dma_start(xsb[:, kb, :], xr[:, kb, :])
                    if kb % w_dma_div == 0:
                        wtmp = wpool.tile([128, NCHUNK], f32, tag="wtmp")
                        nc.sync.dma_start(
                            wtmp[:], WT[ts(kt, 128), ts(c, NCHUNK)])
                    if quant_mode == "full":
                        neg = spool.tile([128, NCHUNK], bf16, tag="neg")
                        nc.vector.tensor_scalar(
                            neg[:], wtmp[:], ntsb[:], None,
                            mybir.AluOpType.is_lt)
                        wqt = wqpool.tile([128, NCHUNK], bf16, tag="wq")
                        nc.vector.scalar_tensor_tensor(
                            wqt[:], wtmp[:], tsb[:], neg[:],
                            mybir.AluOpType.is_gt, mybir.AluOpType.subtract)
                    elif quant_mode == "copy":
                        wqt = wqpool.tile([128, NCHUNK], bf16, tag="wq")
                        nc.vector.tensor_copy(wqt[:], wtmp[:])
                    elif quant_mode == "none":
                        wqt = wtmp
                    if pending:
                        posb, pmt, pc = pending.pop(0)
                        nc.sync.dma_start(
                            out[ts(pmt, 128), ts(pc, NCHUNK)], posb[:])
                    for mt in range(MT // mt_div):
                        nc.tensor.matmul(
                            ps[mt][:], xsb[:, kb, ts(mt, 128)], wqt[:],
                            start=(kt == KT8),
                            stop=(KF == 0 and kt == KT - 1))
                # -- fp8 DoubleRow part: k-slab pairs [2kp, 2kp+1], W pair
                # staged in one [128,2,N] tile so each pair quantizes with
                # a single (neg, stt) DVE op pair over 2N columns. --
                for kp in range(KF):
                    if c == 0:
                        nc.sync.dma_start(xsb8[:, 2 * kp:2 * kp + 2, :],
                                          x8r[:, 2 * kp:2 * kp + 2, :])
                    wp = wppool.tile([128, 2, NCHUNK], f32, tag="wp")
                    for j in range(2):
                        nc.sync.dma_start(
                            wp[:, j, :], WT[ts(2 * kp + j, 128), ts(c, NCHUNK)])
                    negp = spool.tile([128, 2, NCHUNK], bf16, tag="negp")
                    nc.vector.tensor_scalar(
                        negp[:], wp[:], ntsb[:], None, mybir.AluOpType.is_lt)
                    wq8 = wq8pool.tile([128, 2, NCHUNK], f8, tag="wq8")
                    nc.vector.scalar_tensor_tensor(
                        wq8[:], wp[:], tsb[:], negp[:],
                        mybir.AluOpType.is_gt, mybir.AluOpType.subtract)
                    for mt in range(MT // mt_div):
                        nc.tensor.matmul(
                            ps[mt][:], xsb8[:, 2 * kp:2 * kp + 2, ts(mt, 128)],
                            wq8[:], start=(KTB == 0 and kp == 0),
                            stop=(kp == KF - 1),
                            perf_mode=mybir.MatmulPerfMode.DoubleRow)
                for mt in range(MT // mt_div):
                    osb = opool.tile([128, NCHUNK], f32, tag="osb")
                    eng = nc.vector if evict_engine == "vector" else nc.scalar
                    if evict_engine == "vector":
                        eng.tensor_copy(osb[:], ps[mt][:])
                    else:
                        eng.copy(osb[:], ps[mt][:])
                    pending.append((osb, mt, c))
          for posb, pmt, pc in pending:
              nc.sync.dma_start(out[ts(pmt, 128), ts(pc, NCHUNK)], posb[:])

    nc.compile()
    meta = dict(m_core=m_core, k=k, n=n, ncores=ncores, NG=NG)
    return nc, meta


def _get_compiled():
    global _COMPILED
    if _COMPILED is None:
        _COMPILED = build_module()
    return _COMPILED


def make_in_maps(x, W, m_core=M_CORE, ncores=NCORES, fp8_pairs=FP8_PAIRS):
    """Host-side shard prep. x [B,S,D_IN] f32, W [D_OUT,D_IN] f32.
    k < 256*fp8_pairs ships as fp8e4 (consumed by DoubleRow matmuls),
    the rest as bf16."""
    k = W.shape[1]
    n = W.shape[0]
    ng = n // ncores
    k8 = 256 * fp8_pairs
    x2 = np.asarray(x, dtype=np.float32).reshape(-1, k)
    xb = x2[:, k8:].astype(ml_dtypes.bfloat16)
    x8 = x2[:, :k8].astype(ml_dtypes.float8_e4m3)
    WT = np.ascontiguousarray(np.asarray(W, dtype=np.float32).T)  # [k, n]
    in_maps = []
    for c in range(ncores):
        sl = slice(c * m_core, (c + 1) * m_core)
        Wgc = np.ascontiguousarray(WT[:, c * ng:(c + 1) * ng])
        m = {"WT": WT, "Wg": Wgc}
        if k8 < k:
            m["xT"] = np.ascontiguousarray(xb[sl].T)
        if k8 > 0:
            m["xT8"] = np.ascontiguousarray(x8[sl].T)
        in_maps.append(m)
    return in_maps


def kernel(input, W):
    """Full inputs in, full output out. Shards internally across 8 cores."""
    global LAST_RESULTS
    from concourse import bass_utils

    nc, meta = _get_compiled()
    in_maps = make_in_maps(input, W)
    res = bass_utils.run_bass_kernel_spmd(
        nc, in_maps, core_ids=list(range(NCORES)))
    LAST_RESULTS = res
    out = np.concatenate([res.results[c]["out"] for c in range(NCORES)], axis=0)
    return out.reshape(B, S, D_OUT).astype(np.float32)

